# revision 23
# baseline (speedup 1.0000x reference)
# Trainium2 Bass kernel for nn_AdaptiveCrossHadamard.
#
# Reference computation (per sample):
#   y   = BN(Conv1x1(x))                                  [256, 64*64]
#   p   = mean_pixels(y); logits = conv1d(p, eca_w, k=5)  [256]
#   idx = top_32(logits) (sorted desc, ties -> lower idx)
#   xs  = y[idx]                                          [32, 4096]
#   z   = BN_s(xs[hi] * xs[hj])  for all i<j pairs        [496, 4096]
#   out = concat([y, z], channel axis)                    [752, 4096]
#
# Strategy (8 NeuronCores, batch-parallel, 2 samples/core, no collectives):
#   - BN folded into matmul weights host-side; ECA conv1d as a banded 256x256
#     matrix host-side (values from eca_w, structure static).
#   - y via fp16 matmuls (fp32 accumulate in PSUM), bias added by ScalarE on
#     the PSUM->SBUF copy.
#   - pooled computed EXACTLY in f32 via linearity: mean_pix(y) =
#     W' @ sum_pix(x)/4096 + b'  (f32 row-sums of x on VectorE, f32 matmul).
#     This keeps the top-k ranking faithful to the f32 reference.
#   - top-k as dense ops: rank[c] = #{b: logits[b] > logits[c]} (+ exact
#     tie-break via a lower-triangular mask), one-hot selection matrix S from
#     rank==iota, selected channels as a matmul with x (weights = W' @ S).
#   - pairwise Hadamard: one-hot pair matrices replicate the 32 selected rows
#     to 496 pair rows on the TensorEngine; VectorE does
#     t = (A * scale_s) * B in one scalar_tensor_tensor op (A read from PSUM);
#     GpSimd adds the per-pair shift on the way to the output staging tile.
import os
import sys
import numpy as np

_TRN_REPO = "/opt/trn_rl_repo"
if _TRN_REPO not in sys.path and os.path.isdir(_TRN_REPO):
    sys.path.insert(0, _TRN_REPO)

import concourse.bacc as bacc
import concourse.bass as bass
import concourse.mybir as mybir
import concourse.tile as tile
from concourse.bass_utils import run_bass_kernel_spmd

F32 = mybir.dt.float32
F16 = mybir.dt.float16
AF = mybir.ActivationFunctionType
ALU = mybir.AluOpType

B, C1, H, W = 16, 256, 64, 64
PIX = H * W                      # 4096
CS = 32
CSE = CS * (CS - 1) // 2         # 496
NCORES = 8
SPC = B // NCORES                # samples per core = 2
COUT = C1 + CSE                  # 752
EPS = 1e-5

NT = PIX // 512                  # 8 pixel tiles of 512
MT4 = (CSE + 127) // 128         # 4 pair-row tiles (128,128,128,112)


def _build(nc: bass.Bass, dbg: bool = False):
    """Emit the per-core Tile program. SPMD: all 8 cores run this graph."""
    x_d = nc.dram_tensor("x", [SPC * C1, PIX], F32, kind="ExternalInput")
    out_d = nc.dram_tensor("out", [SPC * COUT, PIX], F32, kind="ExternalOutput")
    if dbg:
        dbg_pooled = nc.dram_tensor("dbg_pooled", [SPC * C1, 1], F32,
                                    kind="ExternalOutput")
        dbg_lrow = nc.dram_tensor("dbg_lrow", [SPC, C1], F32,
                                  kind="ExternalOutput")
        dbg_rank = nc.dram_tensor("dbg_rank", [SPC * C1, 1], F32,
                                  kind="ExternalOutput")
        dbg_st = nc.dram_tensor("dbg_st", [SPC * C1, CS], F16,
                                kind="ExternalOutput")
        dbg_xsel = nc.dram_tensor("dbg_xsel", [SPC * CS, PIX], F16,
                                  kind="ExternalOutput")
        dbg_t = nc.dram_tensor("dbg_t", [SPC * 512, PIX], F32,
                               kind="ExternalOutput")

    wyT16_d = nc.dram_tensor("wyT16", [C1, C1], F16, kind="ExternalInput")
    wyT32s_d = nc.dram_tensor("wyT32s", [C1, C1], F32, kind="ExternalInput")
    wfold16_d = nc.dram_tensor("wfold16", [C1, C1], F16, kind="ExternalInput")
    bcol_d = nc.dram_tensor("bcol", [C1, 1], F32, kind="ExternalInput")
    bcol16_d = nc.dram_tensor("bcol16", [C1, 1], F16, kind="ExternalInput")
    cmat_d = nc.dram_tensor("cmat", [C1, C1], F32, kind="ExternalInput")
    tril_d = nc.dram_tensor("tril", [C1, C1], F32, kind="ExternalInput")
    offd_d = nc.dram_tensor("offd", [C1, C1], F32, kind="ExternalInput")
    iota_d = nc.dram_tensor("iota32", [128, CS], F32, kind="ExternalInput")
    piT_d = nc.dram_tensor("piT", [CS, CSE], F16, kind="ExternalInput")
    pjT_d = nc.dram_tensor("pjT", [CS, CSE], F16, kind="ExternalInput")
    scol_d = nc.dram_tensor("scol", [CSE, 1], F32, kind="ExternalInput")
    shcol_d = nc.dram_tensor("shcol", [CSE, 1], F32, kind="ExternalInput")

    from contextlib import ExitStack
    with tile.TileContext(nc) as tc, ExitStack() as ctx:
        cpool = ctx.enter_context(tc.tile_pool(name="consts", bufs=1))
        x32p = ctx.enter_context(tc.tile_pool(name="x32", bufs=2))
        x16p = ctx.enter_context(tc.tile_pool(name="x16", bufs=4))
        yp = ctx.enter_context(tc.tile_pool(name="ysb", bufs=2))
        tp = ctx.enter_context(tc.tile_pool(name="tsb", bufs=2))
        zp = ctx.enter_context(tc.tile_pool(name="zout", bufs=2))
        b16p = ctx.enter_context(tc.tile_pool(name="b16", bufs=4))
        xselp = ctx.enter_context(tc.tile_pool(name="xsel", bufs=2))
        gp = ctx.enter_context(tc.tile_pool(name="gwork", bufs=2))
        smallp = ctx.enter_context(tc.tile_pool(name="small", bufs=4))
        # PSUM partitioned: y/xsel/small matmuls can't be starved by the
        # pair-phase A/B traffic of the other sample.
        psMM = ctx.enter_context(tc.tile_pool(name="psMM", bufs=3, space="PSUM"))
        psA = ctx.enter_context(tc.tile_pool(name="psA", bufs=3, space="PSUM"))
        psB = ctx.enter_context(tc.tile_pool(name="psB", bufs=2, space="PSUM"))

        dma = nc.sync.dma_start

        # ---- load constants (unique tag per tile: they live forever) ----
        _cn = [0]

        def cload(dram, shape, dtype, row0=0):
            _cn[0] += 1
            t = cpool.tile(list(shape), dtype, tag=f"c{_cn[0]}")
            dma(out=t[:], in_=dram[row0:row0 + shape[0], :])
            return t

        wyT16 = [cload(wyT16_d, (128, C1), F16, k * 128) for k in range(2)]
        wyT32s = [cload(wyT32s_d, (128, C1), F32, k * 128) for k in range(2)]
        wfold16 = [cload(wfold16_d, (128, C1), F16, k * 128) for k in range(2)]
        cmat = [cload(cmat_d, (128, C1), F32, k * 128) for k in range(2)]
        tril = [cload(tril_d, (128, C1), F32, k * 128) for k in range(2)]
        offd = [cload(offd_d, (128, C1), F32, k * 128) for k in range(2)]
        bcol = [cload(bcol_d, (128, 1), F32, k * 128) for k in range(2)]
        bcol16 = [cload(bcol16_d, (128, 1), F16, k * 128) for k in range(2)]
        iota32 = cload(iota_d, (128, CS), F32)
        piT = cload(piT_d, (CS, CSE), F16)
        pjT = cload(pjT_d, (CS, CSE), F16)
        scol, shcol = [], []
        for m in range(MT4):
            p = min(128, CSE - m * 128)
            scol.append(cload(scol_d, (p, 1), F32, m * 128))
            shcol.append(cload(shcol_d, (p, 1), F32, m * 128))

        X16 = [None] * SPC
        XSUM = [None] * SPC
        POOLED = [None] * SPC
        WSEL = [None] * SPC
        SBIAS = [None] * SPC
        XSEL = [None] * SPC

        def ph_load(s):
            # load x, cast fp16 + exact f32 row-sums in one DVE pass
            X16[s], XSUM[s] = [], []
            for kt in range(2):
                x32 = x32p.tile([128, PIX], F32)
                dma(out=x32[:],
                    in_=x_d[s * C1 + kt * 128: s * C1 + (kt + 1) * 128, :])
                xs = smallp.tile([128, 1], F32, tag="xsum")
                xt = x16p.tile([128, PIX], F16)
                nc.vector.tensor_scalar(xt[:], x32[:], 1.0, 0.0, op0=ALU.mult,
                                        op1=ALU.add, accum_out=xs[:])
                X16[s].append(xt)
                XSUM[s].append(xs)

        def ph_y(s):
            # y = W'x + b' (fp16 matmul, f32 psum), ACT adds bias
            for mt in range(2):
                y_sb = yp.tile([128, PIX], F32)
                for nt in range(NT):
                    y_ps = psMM.tile([128, 512], F32, tag="mm")
                    for kt in range(2):
                        nc.tensor.matmul(
                            y_ps[:],
                            lhsT=wyT16[kt][:, mt * 128:(mt + 1) * 128],
                            rhs=X16[s][kt][:, nt * 512:(nt + 1) * 512],
                            start=(kt == 0), stop=(kt == 1))
                    nc.scalar.activation(
                        y_sb[:, nt * 512:(nt + 1) * 512], y_ps[:],
                        AF.Identity, bias=bcol[mt][:], scale=1.0)
                dma(out=out_d[s * COUT + mt * 128: s * COUT + (mt + 1) * 128, :],
                    in_=y_sb[:])

        def ph_sel(s):
            # pooled = W'@xbar + b' (exact f32; wyT32s folds the /4096)
            pooled = []
            for mt in range(2):
                pp = psMM.tile([128, 1], F32, tag="mm")
                for kt in range(2):
                    nc.tensor.matmul(
                        pp[:], lhsT=wyT32s[kt][:, mt * 128:(mt + 1) * 128],
                        rhs=XSUM[s][kt][:], start=(kt == 0), stop=(kt == 1))
                pb = smallp.tile([128, 1], F32, tag="pooled")
                nc.scalar.activation(pb[:], pp[:], AF.Identity,
                                     bias=bcol[mt][:], scale=1.0)
                pooled.append(pb)
                if dbg:
                    dma(out=dbg_pooled[s * C1 + mt * 128:
                                       s * C1 + (mt + 1) * 128, :], in_=pb[:])
            POOLED[s] = pooled

            lr_ps = psMM.tile([1, C1], F32, tag="mm")
            for ot in range(2):
                nc.tensor.matmul(lr_ps[:], lhsT=pooled[ot][:], rhs=cmat[ot][:],
                                 start=(ot == 0), stop=(ot == 1))
            lrow = smallp.tile([1, C1], F32, tag="lrow")
            nc.scalar.copy(lrow[:], lr_ps[:])
            if dbg:
                dma(out=dbg_lrow[s:s + 1, :], in_=lrow[:])

            st = []
            for qt in range(2):
                lc_ps = psMM.tile([128, 1], F32, tag="mm")
                for ot in range(2):
                    nc.tensor.matmul(
                        lc_ps[:], lhsT=cmat[ot][:, qt * 128:(qt + 1) * 128],
                        rhs=pooled[ot][:], start=(ot == 0), stop=(ot == 1))
                lcol = smallp.tile([128, 1], F32, tag="lcol")
                nc.scalar.copy(lcol[:], lc_ps[:])

                # exact broadcast of logits row to all partitions (no PE fp32
                # rounding: the fp32 PE path is ~1e-7 lossy, which made the
                # diagonal compare Brow[a,a] vs lcol[a] misfire)
                brow = gp.tile([128, C1], F32, tag="brow")
                nc.gpsimd.partition_broadcast(brow[:], lrow[:])
                # rank[a] = #{b!=a: logits[b] > logits[a]}
                #        + #{b < a: logits[b] == logits[a]}   (jax tie-break)
                g2 = gp.tile([128, C1], F32)
                nc.vector.scalar_tensor_tensor(
                    g2[:], brow[:], lcol[:], tril[qt][:],
                    op0=ALU.is_equal, op1=ALU.mult)
                gsum = gp.tile([128, C1], F32)
                nc.vector.scalar_tensor_tensor(
                    gsum[:], brow[:], lcol[:], g2[:],
                    op0=ALU.is_gt, op1=ALU.add)
                gm = gp.tile([128, C1], F32)
                nc.vector.tensor_tensor(gm[:], gsum[:], offd[qt][:],
                                        op=ALU.mult)
                rank = smallp.tile([128, 1], F32, tag="rank")
                nc.vector.tensor_reduce(rank[:], gm[:],
                                        axis=mybir.AxisListType.X, op=ALU.add)
                # S_T[c, k] = (rank[c] == k)
                stq = smallp.tile([128, CS], F16, tag="st")
                nc.vector.tensor_scalar(stq[:], iota32[:], rank[:], None,
                                        op0=ALU.is_equal)
                st.append(stq)
                if dbg:
                    r0 = s * C1 + qt * 128
                    dma(out=dbg_rank[r0:r0 + 128, :], in_=rank[:])
                    dma(out=dbg_st[r0:r0 + 128, :], in_=stq[:])

            # selection weights: W_selT[c,k] = sum_o W'[o,c] S_T[o,k]
            wsel = []
            for ct in range(2):
                ws_ps = psMM.tile([128, CS], F32, tag="mm")
                for ot in range(2):
                    nc.tensor.matmul(
                        ws_ps[:], lhsT=wfold16[ot][:, ct * 128:(ct + 1) * 128],
                        rhs=st[ot][:], start=(ot == 0), stop=(ot == 1))
                wsq = smallp.tile([128, CS], F16, tag="wsel")
                nc.scalar.copy(wsq[:], ws_ps[:])
                wsel.append(wsq)
            WSEL[s] = wsel
            sb_ps = psMM.tile([CS, 1], F32, tag="mm")
            for ot in range(2):
                nc.tensor.matmul(sb_ps[:], lhsT=st[ot][:], rhs=bcol16[ot][:],
                                 start=(ot == 0), stop=(ot == 1))
            sbias = smallp.tile([CS, 1], F32, tag="sbias")
            nc.scalar.copy(sbias[:], sb_ps[:])
            SBIAS[s] = sbias

        def ph_xsel(s):
            # x_sel = W_sel @ x + S b'  (fp16, straight from x)
            xsel = xselp.tile([CS, PIX], F16)
            for nt in range(NT):
                xs_ps = psMM.tile([CS, 512], F32, tag="mm")
                for kt in range(2):
                    nc.tensor.matmul(
                        xs_ps[:], lhsT=WSEL[s][kt][:],
                        rhs=X16[s][kt][:, nt * 512:(nt + 1) * 512],
                        start=(kt == 0), stop=(kt == 1))
                nc.scalar.activation(xsel[:, nt * 512:(nt + 1) * 512], xs_ps[:],
                                     AF.Identity, bias=SBIAS[s][:], scale=1.0)
            XSEL[s] = xsel
            if dbg:
                dma(out=dbg_xsel[s * CS:(s + 1) * CS, :], in_=xsel[:])

        def ph_z(s, m, half):
            # pairwise Hadamard + BN_s for pair-rows [m*128, m*128+p) and
            # pixel half `half` (2048 columns)
            p = min(128, CSE - m * 128)
            xsel = XSEL[s]
            t_sb = tp.tile([128, PIX // 2], F32)
            for j in range(NT // 2):
                nt = half * (NT // 2) + j
                a_ps = psA.tile([128, 512], F32, tag="a")
                b_ps = psB.tile([128, 512], F32, tag="b")
                nc.tensor.matmul(a_ps[:p, :],
                                 lhsT=piT[:, m * 128: m * 128 + p],
                                 rhs=xsel[:, nt * 512:(nt + 1) * 512],
                                 start=True, stop=True)
                nc.tensor.matmul(b_ps[:p, :],
                                 lhsT=pjT[:, m * 128: m * 128 + p],
                                 rhs=xsel[:, nt * 512:(nt + 1) * 512],
                                 start=True, stop=True)
                b16 = b16p.tile([128, 512], F16)
                nc.scalar.copy(b16[:p, :], b_ps[:p, :])
                # t = (A * scale_s) * B
                nc.vector.scalar_tensor_tensor(
                    t_sb[:p, j * 512:(j + 1) * 512],
                    a_ps[:p, :], scol[m][:], b16[:p, :],
                    op0=ALU.mult, op1=ALU.mult)
            if dbg:
                dma(out=dbg_t[s * 512 + m * 128: s * 512 + m * 128 + p,
                              half * (PIX // 2):(half + 1) * (PIX // 2)],
                    in_=t_sb[:p, :])
            zo = zp.tile([128, PIX // 2], F32)
            nc.vector.tensor_scalar(zo[:p, :], t_sb[:p, :], shcol[m][:],
                                    None, op0=ALU.add)
            r0 = s * COUT + C1 + m * 128
            dma(out=out_d[r0:r0 + p,
                          half * (PIX // 2):(half + 1) * (PIX // 2)],
                in_=zo[:p, :])

        # program order == scheduling priority: selection chains early,
        # y matmuls fill gaps, z phases of both samples interleaved.
        ph_load(0)
        ph_load(1)
        ph_sel(0)
        ph_y(0)
        ph_sel(1)
        ph_xsel(0)
        ph_y(1)
        ph_xsel(1)
        for m in range(MT4):
            for half in range(2):
                ph_z(0, m, half)
                ph_z(1, m, half)

    return nc


_CACHE = {}


def _get_nc(dbg: bool = False):
    key = f"nc{int(dbg)}"
    if key not in _CACHE:
        nc = bacc.Bacc("TRN2", target_bir_lowering=False, debug=False,
                       num_devices=NCORES)
        _build(nc, dbg=dbg)
        nc.compile()
        _CACHE[key] = nc
    return _CACHE[key]


def _host_params(w_fc, b_fc, g_x, b_x, m_x, v_x, eca_w, g_s, b_s, m_s, v_s):
    sx = (g_x / np.sqrt(v_x + EPS)).astype(np.float32)            # [256]
    Wp = (sx[:, None] * w_fc).astype(np.float32)                  # [o, c]
    bp = (sx * b_fc + b_x - m_x * sx).astype(np.float32)          # [256]

    cmat = np.zeros((C1, C1), np.float32)                         # [o, q]
    for k in range(5):
        d = k - 2                                                 # o - q
        for q in range(C1):
            o = q + d
            if 0 <= o < C1:
                cmat[o, q] = eca_w[k]

    tril = (np.arange(C1)[None, :] < np.arange(C1)[:, None]).astype(np.float32)

    hi, hj = np.triu_indices(CS, k=1)
    piT = np.zeros((CS, CSE), np.float16)
    pjT = np.zeros((CS, CSE), np.float16)
    piT[hi, np.arange(CSE)] = 1.0
    pjT[hj, np.arange(CSE)] = 1.0

    ss = (g_s / np.sqrt(v_s + EPS)).astype(np.float32)
    sh = (b_s - m_s * ss).astype(np.float32)

    return {
        "wyT16": Wp.T.astype(np.float16).copy(),
        "wyT32s": (Wp.T / PIX).astype(np.float32).copy(),
        "wfold16": Wp.astype(np.float16).copy(),
        "bcol": bp.reshape(C1, 1).copy(),
        "bcol16": bp.astype(np.float16).reshape(C1, 1).copy(),
        "cmat": cmat,
        "tril": tril,
        "offd": (1.0 - np.eye(C1, dtype=np.float32)),
        "iota32": np.tile(np.arange(CS, dtype=np.float32), (128, 1)).copy(),
        "piT": piT,
        "pjT": pjT,
        "scol": ss.reshape(CSE, 1).copy(),
        "shcol": sh.reshape(CSE, 1).copy(),
    }


def _in_maps(inputs):
    x = np.ascontiguousarray(np.asarray(inputs["x"], np.float32))
    params = _host_params(
        np.asarray(inputs["w_fc"], np.float32),
        np.asarray(inputs["b_fc"], np.float32),
        np.asarray(inputs["bn_x_gamma"], np.float32),
        np.asarray(inputs["bn_x_beta"], np.float32),
        np.asarray(inputs["bn_x_mean"], np.float32),
        np.asarray(inputs["bn_x_var"], np.float32),
        np.asarray(inputs["eca_w"], np.float32),
        np.asarray(inputs["bn_s_gamma"], np.float32),
        np.asarray(inputs["bn_s_beta"], np.float32),
        np.asarray(inputs["bn_s_mean"], np.float32),
        np.asarray(inputs["bn_s_var"], np.float32),
    )
    maps = []
    for c in range(NCORES):
        shard = x[c * SPC:(c + 1) * SPC].reshape(SPC * C1, PIX)
        maps.append({"x": np.ascontiguousarray(shard), **params})
    return maps


def _ensure_ntff_hook():
    """The agent image lacks antenv.axon_hooks; synthesize it so
    run_bass_kernel_spmd(trace=True) can reach the NTFF profiler in
    libaxon_pjrt.so. Safe no-op if anything is missing."""
    try:
        import antenv.axon_hooks  # noqa: F401
        return
    except ImportError:
        pass
    try:
        import types
        import antenv
        from trn_agent_boot.trn_boot import _ntff_profile_via_ctypes
        hook = _ntff_profile_via_ctypes("/opt/axon/libaxon_pjrt.so")
        mod = types.ModuleType("antenv.axon_hooks")
        mod._hook = hook
        mod.get_axon_ntff_profile_hook = lambda: mod._hook
        mod.set_axon_ntff_profile_hook = lambda h: setattr(mod, "_hook", h)
        sys.modules["antenv.axon_hooks"] = mod
        antenv.axon_hooks = mod
    except Exception as e:  # pragma: no cover
        print(f"ntff hook shim failed: {e}", file=sys.stderr)


def run(inputs, trace=False, dbg=False):
    if trace:
        _ensure_ntff_hook()
    nc = _get_nc(dbg=dbg)
    maps = _in_maps(inputs)
    res = run_bass_kernel_spmd(nc, maps, core_ids=list(range(NCORES)),
                               trace=trace)
    outs = [np.asarray(res.results[c]["out"], np.float32)
            .reshape(SPC, COUT, H, W) for c in range(NCORES)]
    return np.concatenate(outs, axis=0), res


def kernel(**inputs) -> np.ndarray:
    out, _ = run(inputs, trace=False)
    return out
